# revision 1
# baseline (speedup 1.0000x reference)
"""ALiBi multi-head attention on 8 TRN2 NeuronCores (~164us HW exec).

Problem: x [2, 2048, 1024] fp32, W_kqv [3072, 1024] fp32 (row chunks k,q,v),
16 heads x 64 dim, causal + ALiBi, softmax scale = sqrt(1024) = 32.

Sharding (no collectives): core c takes batch c//4 and the STRIDED heads
{hb, hb+4, hb+8, hb+12} with hb = c%4. Striding makes each local head
slot's ALiBi slope range uniform across cores, so the shared SPMD graph
can skip score tiles whose ALiBi decay makes them negligible
(slope*distance >= 12, ~1e-6 of softmax mass) - 47/160 tiles dropped.

Device design (per core):
- Host pre-transposes x and the per-head W column shards and pre-casts to
  bf16, so every matmul contracts over the partition dim with no on-device
  transposes or casts; the 1/32 score scale is folded into the Q weights.
- Q^T/K^T land in [d, s] layout, one padded [128, S] tile per head (data
  in one 64-partition half). Scores are computed transposed, S^T[j, i],
  so softmax(j) runs along partitions: no max-subtraction needed
  (causal+ALiBi bound scores above by ~2) and the 2048^2 probability
  matrix is never transposed.
- The ALiBi bias slope*(j-i) is folded into the score matmul itself:
  four bf16 bias rows in the padded partitions form exact rank-1 pairs
  slope*((j mod 128) - (i mod 256)) (integers <= 255 are bf16-exact;
  slope split sH+sL keeps products exact in fp32 MACs); the remaining
  per-(kt, column-group) constant enters through exp's per-partition
  bias AP (fp32, data-driven, SPMD-safe). exp reads scores straight from
  PSUM; no per-tile vector-engine work remains.
- Causal masking of diagonal tiles is a GpSimd affine_select zero-fill on
  the exp output (masked entries overflow to +inf; the fill never reads
  them), and their exp skips the fully-masked column prefix.
- The denominator comes from a ones column appended to V (one extra PSUM
  row in the PV matmul). The raw [65, 512] accumulators (outputs +
  denominator row) ship to DRAM; the host folds the normalize and the
  [d, q] -> [q, d] transpose into the gather.
- Emission interleaves the QKV projection with attention blocks as their
  dependencies complete, so the in-order engine queues have no phase
  barrier; four heads interleave per (qc, kt) to keep independent matmuls
  ready around each head's softmax chain.
- All matmuls are bf16 with fp32 PSUM accumulation at full K=128
  contraction - the fastest PE path that keeps the HAM clock-gate warm
  (fp32/f32r matmuls run in transpose-mode, which the activity monitor
  ignores, and the PE then sticks at 1.2 GHz).
"""

import math
import os
import sys

import numpy as np

for _p in ("/opt/trn_rl_repo",):
    if _p not in sys.path:
        sys.path.insert(0, _p)

B, S, E = 2, 2048, 1024
H, D = 16, 64
H_LOC = 4          # heads per core
COLS = H_LOC * D   # 256 output columns per core
SCALE = 1.0 / math.sqrt(E)
N_CORES = 8

_NC_CACHE = [None]


def _build():
    import concourse.bacc as bacc
    import concourse.mybir as mybir
    import concourse.tile as tile

    f32 = mybir.dt.float32
    bf16 = mybir.dt.bfloat16
    nc = bacc.Bacc("TRN2", target_bir_lowering=False, debug=False,
                   num_devices=N_CORES)

    xt = nc.dram_tensor("xt", [E, S], mybir.dt.bfloat16,
                        kind="ExternalInput")
    wt_qk = nc.dram_tensor("wt_qk", [E, 2 * COLS], mybir.dt.bfloat16,
                           kind="ExternalInput")
    wt_v = nc.dram_tensor("wt_v", [E, COLS], mybir.dt.bfloat16,
                          kind="ExternalInput")
    slopes = nc.dram_tensor("slopes", [128, H_LOC], f32, kind="ExternalInput")
    brows_k = nc.dram_tensor("brows_k", [4 * H_LOC, S], mybir.dt.bfloat16,
                             kind="ExternalInput")
    brows_q = nc.dram_tensor("brows_q", [4 * H_LOC, S], mybir.dt.bfloat16,
                             kind="ExternalInput")
    out = nc.dram_tensor("out", [H_LOC * 65, S], f32,
                         kind="ExternalOutput")

    NE = E // 128     # 8 e-tiles
    NS = S // 512     # 4 s-chunks of 512
    NST = S // 128    # 16 s-tiles of 128

    with tile.TileContext(nc) as tc:
        with tc.tile_pool(name="const", bufs=1) as cpool, \
             tc.tile_pool(name="persist", bufs=1) as pp, \
             tc.tile_pool(name="work", bufs=8) as wp, \
             tc.tile_pool(name="ps_s", bufs=4, space="PSUM") as ps_s:

            # ---- constants ----
            slp = cpool.tile([128, H_LOC], f32, tag="slp")
            ones4 = cpool.tile([128, H_LOC, 1], f32, tag="ones4")
            nc.vector.memset(ones4[:], 1.0)

            # Per-(head, delta) exp-bias columns: CB[:, h*32 + delta+16] =
            # slope_h * 128 * delta, the coarse part of the ALiBi bias
            # (delta = kt - 2g for 256-wide column group g). The fine part
            # rides inside the score matmul as exact bf16 rank-2 pairs.
            dramp = cpool.tile([128, 32], f32, tag="dramp")
            nc.gpsimd.iota(dramp[:], pattern=[[128, 32]], base=-2048,
                           channel_multiplier=0,
                           allow_small_or_imprecise_dtypes=True)
            cb = cpool.tile([128, H_LOC * 32], f32, tag="cb")
            for h in range(H_LOC):
                nc.vector.tensor_scalar_mul(
                    cb[:, h * 32:(h + 1) * 32], dramp[:], slp[:, h:h + 1])

            # ---- persistent activations ----
            # Per-head Q^T/K^T [128, S] with the unused 64-partition half
            # zeroed: keeps every score matmul at full K=128 contraction
            # (zeros contribute nothing; matmul cost is N cycles either way)
            # so the PE activity monitor sees a fully-busy array.
            qt = [pp.tile([128, S], bf16, tag=f"qt{h}", name=f"qt{h}")
                  for h in range(H_LOC)]
            kt_t = [pp.tile([128, S], bf16, tag=f"kt{h}", name=f"ktt{h}")
                    for h in range(H_LOC)]
            for h in range(H_LOC):
                # zero the whole non-data half (32-aligned), then lay the 4
                # bias rows over it (Tile orders the overlapping writes)
                br = 64 if h % 2 == 0 else 60
                z0 = 64 if h % 2 == 0 else 0
                # heads 0/1 gate early attention: zero them on the DVE
                # (fast); heads 2/3 zero on the idle GpSimd in parallel
                eng = nc.vector if h < 2 else nc.gpsimd
                eng.memset(qt[h][z0:z0 + 64, :], 0.0)
                eng.memset(kt_t[h][z0:z0 + 64, :], 0.0)
            va = [pp.tile([128, H_LOC * 65], bf16, tag=f"va{st}", name=f"va{st}")
                  for st in range(NST)]

            # ---- phase 1 + 2, interleaved emission ----
            # Inputs arrive pre-cast to bf16 (host-side; identical numerics
            # to a device cast) - half the DMA bytes and no cast ops.
            # Attention blocks are emitted as soon as their dependencies
            # exist (QK tiles for the heads, V tiles up to the block's
            # ktmax), so the in-order engine queues have no phase barrier
            # and the Scalar engine's exp stream starts ~40us early.
            with tc.tile_pool(name="inp", bufs=1) as ip, \
                 tc.tile_pool(name="ps_o", bufs=4, space="PSUM") as ps_o:
                xtr = [ip.tile([128, S], bf16, tag=f"xt{e}", name=f"xtr{e}")
                       for e in range(NE)]
                wqk = [ip.tile([128, 2 * COLS], bf16, tag=f"wqk{e}",
                               name=f"wqk{e}") for e in range(NE)]
                wv = [ip.tile([128, COLS], bf16, tag=f"wv{e}", name=f"wv{e}")
                      for e in range(NE)]
                nc.sync.dma_start(slp[:], slopes[:, :])
                for e in range(NE):
                    nc.sync.dma_start(xtr[e][:], xt[e * 128:(e + 1) * 128, :])
                    nc.sync.dma_start(wqk[e][:],
                                      wt_qk[e * 128:(e + 1) * 128, :])
                    nc.sync.dma_start(wv[e][:],
                                      wt_v[e * 128:(e + 1) * 128, :])
                for h in range(H_LOC):
                    br = 64 if h % 2 == 0 else 60
                    nc.sync.dma_start(kt_t[h][br:br + 4, :],
                                      brows_k[4 * h:4 * h + 4, :])
                    nc.sync.dma_start(qt[h][br:br + 4, :],
                                      brows_q[4 * h:4 * h + 4, :])

                # Q^T / K^T: [f, s] layout. f-tiles 0,1 = Q heads (01)(23);
                # 2,3 = K heads. The 1/32 score scale is folded into the Q
                # weights host-side. Each psum half-row block goes to its
                # head's padded tile (same partitions - engines cannot move
                # data across partitions).
                def qk_tiles(f):
                    for sc in range(NS):
                        p = ps_s.tile([128, 512], f32, tag="s", name="pqk")
                        for e in range(NE):
                            nc.tensor.matmul(
                                p[:],
                                wqk[e][:, f * 128:(f + 1) * 128],
                                xtr[e][:, sc * 512:(sc + 1) * 512],
                                start=(e == 0), stop=(e == NE - 1))
                        sl = slice(sc * 512, (sc + 1) * 512)
                        dst = qt if f < 2 else kt_t
                        fb = f if f < 2 else f - 2
                        nc.vector.tensor_copy(dst[2 * fb][0:64, sl],
                                              p[0:64, :])
                        nc.vector.tensor_copy(dst[2 * fb + 1][64:128, sl],
                                              p[64:128, :])

                # V in [s, d] layout, augmented with a ones column per head.
                def v_tiles(st0, st1):
                    for st in range(st0, st1):
                        p = ps_s.tile([128, COLS], f32, tag="s", name="pv",
                                      padded_shape=[128, 512])
                        for e in range(NE):
                            nc.tensor.matmul(
                                p[:],
                                xtr[e][:, st * 128:(st + 1) * 128],
                                wv[e][:],
                                start=(e == 0), stop=(e == NE - 1))
                        var = va[st][:].rearrange("p (h c) -> p h c", h=H_LOC)
                        nc.vector.tensor_copy(
                            var[:, :, 0:64],
                            p[:].rearrange("p (h c) -> p h c", h=H_LOC))
                        nc.vector.tensor_copy(var[:, :, 64:65], ones4[:])

                def attn_score(h, qc, kt):
                    # score matmul carries the fine ALiBi term in its bias
                    # rows; the coarse per-(kt, column-group) constant
                    # enters via the exp's per-partition bias AP.
                    # columns beyond hi_clip are past the ALiBi skip
                    # threshold for every partition of this tile: compute,
                    # exp and stream only the live columns (same
                    # negligibility bound as whole-tile skipping); the
                    # trailing psum region is never read
                    hi_clip = min(512, kt * 128 + 127 + DJ[h] - qc * 512 + 1)
                    ps = ps_s.tile([128, 512], f32, tag="s", name="ps")
                    nc.tensor.matmul(
                        ps[:, 0:hi_clip],
                        kt_t[h][:, kt * 128:(kt + 1) * 128],
                        qt[h][:, qc * 512:qc * 512 + hi_clip],
                        start=True, stop=True)
                    et = wp.tile([128, 512], bf16, tag="et", name="et")
                    d = kt - 4 * qc
                    for half in range(2):
                        # diagonal tiles: columns below the causal staircase
                        # are filled with 0 by the affine_select; skip exp
                        lo = max(128 * d if d >= 0 else 0, half * 256)
                        hi = min((half + 1) * 256, hi_clip)
                        if lo >= hi:
                            continue
                        delta = kt - 2 * (qc * 2 + half)
                        nc.scalar.activation(
                            et[:, lo:hi], ps[:, lo:hi],
                            mybir.ActivationFunctionType.Exp,
                            bias=cb[:, h * 32 + delta + 16:
                                    h * 32 + delta + 17])
                    if hi_clip < 512:
                        nc.vector.memset(et[:, hi_clip:512], 0.0)
                    if d >= 0:
                        # zero the causally-masked staircase (exp overflowed
                        # to +inf there; the fill never reads it). The fill
                        # region f < p + 128d lies inside the first
                        # 128(d+1) columns, so clip the op to shorten the
                        # exp -> mask -> PV chain.
                        w = min(512, 128 * (d + 1))
                        nc.gpsimd.affine_select(
                            out=et[:, 0:w], in_=et[:, 0:w],
                            compare_op=mybir.AluOpType.is_ge,
                            fill=0.0, base=-128 * d, pattern=[[1, w]],
                            channel_multiplier=-1)
                    return et, hi_clip

                def attn_tile(h, qc, kt, po, ktmax, ktmin):
                    et, hic = attn_score(h, qc, kt)
                    # clipped tiles: trailing et columns are zero; stream
                    # only the live columns. The group's last matmul (the
                    # diagonal tile) is always full width, so every psum
                    # column is written at least once.
                    nc.tensor.matmul(
                        po[:, 0:hic], va[kt][:, h * 65:(h + 1) * 65],
                        et[:, 0:hic],
                        start=(kt == ktmin), stop=(kt == ktmax))

                def attn_epilogue(h, qc, po):
                    # ship the raw accumulator (64 output rows + denominator
                    # row); the host folds the normalize + [d,q]->[q,d]
                    # transpose into the gather.
                    osb = wp.tile([65, 512], f32, tag="osb", name="osb")
                    nc.vector.tensor_copy(osb[:], po[:])
                    # two column-halves on separate DMA queues: the final
                    # epilogue's store is serial tail, and one queue moves
                    # 133KB in ~6us
                    for c in range(2):
                        nc.sync.dma_start(
                            out[h * 65:(h + 1) * 65,
                                qc * 512 + c * 256:qc * 512 + (c + 1) * 256],
                            osb[:, c * 256:(c + 1) * 256])

                # ALiBi tile-skip thresholds per local head slot (strided
                # head assignment keeps slot slope ranges uniform across
                # cores, so the shared graph may skip these tiles)
                DJ = [12 * (4 ** (j + 1)) for j in range(H_LOC)]

                def kt_min(h, qc):
                    for kt in range(16):
                        if qc * 512 - kt * 128 - 127 < DJ[h]:
                            return kt
                    return 16

                def attn_block(qc, heads):
                    attn_block2([(qc, h) for h in heads])

                def attn_block2(streams):
                    # streams: list of (qc, h), up to 4 (one PSUM
                    # accumulator each); interleaved per kt for ILP
                    info = []
                    pos = {}
                    for qc, h in streams:
                        ktmax = (qc * 512 + 511) // 128
                        km = kt_min(h, qc)
                        pos[(qc, h)] = ps_o.tile([65, 512], f32, tag="o",
                                                 name=f"po{qc}{h}")
                        info.append((qc, h, km, ktmax))
                    for kt in range(16):
                        for qc, h, km, ktmax in info:
                            if km <= kt <= ktmax:
                                attn_tile(h, qc, kt, pos[(qc, h)], ktmax, km)
                    for qc, h, km, ktmax in info:
                        attn_epilogue(h, qc, pos[(qc, h)])

                qk_tiles(0)
                qk_tiles(2)
                v_tiles(0, 8)
                attn_block2([(0, 0), (0, 1), (1, 0), (1, 1)])
                qk_tiles(1)
                qk_tiles(3)
                v_tiles(8, 12)
                attn_block2([(0, 2), (0, 3), (1, 2), (1, 3)])
                v_tiles(12, 16)
                # cross-qc stream grouping: the four short slot-0/1
                # streams of qc2+qc3 run together, then the four long
                # slot-2/3 streams - 4-way ILP persists to the end
                attn_block2([(2, 0), (2, 1), (3, 0), (3, 1)])
                attn_block2([(2, 2), (2, 3), (3, 2), (3, 3)])

    nc.compile()
    return nc


def _get_nc():
    if _NC_CACHE[0] is None:
        _NC_CACHE[0] = _build()
    return _NC_CACHE[0]


def _alibi_slopes():
    x = (2 ** 8) ** (1.0 / H)
    return np.array([1.0 / x ** (i + 1) for i in range(H)], dtype=np.float32)


def _bias_row_blocks(slopes4: np.ndarray):
    """bf16 bias rows for the score matmuls (per local head h, 4 rows each).

    K side rows: [m, sH, m, sL]; Q side rows: [sH, -r, sL, -r] with
    m = j mod 128, r = i mod 256 (bf16-exact integers) and
    slope = sH + sL split across two bf16 values so every product in the
    matmul is exact in fp32.
    """
    import ml_dtypes
    m = (np.arange(S) % 128).astype(np.float32)
    r = (np.arange(S) % 256).astype(np.float32)
    bk = np.zeros((4 * H_LOC, S), dtype=np.float32)
    bq = np.zeros((4 * H_LOC, S), dtype=np.float32)
    for h in range(H_LOC):
        sh = np.float32(ml_dtypes.bfloat16(slopes4[h]))
        sl = np.float32(ml_dtypes.bfloat16(np.float32(slopes4[h]) - sh))
        bk[4 * h + 0] = m
        bk[4 * h + 1] = sh
        bk[4 * h + 2] = m
        bk[4 * h + 3] = sl
        bq[4 * h + 0] = sh
        bq[4 * h + 1] = -r
        bq[4 * h + 2] = sl
        bq[4 * h + 3] = -r
    return (bk.astype(ml_dtypes.bfloat16), bq.astype(ml_dtypes.bfloat16))


def kernel(x: np.ndarray, W_kqv: np.ndarray) -> np.ndarray:
    from concourse.bass_utils import run_bass_kernel_spmd

    x = np.asarray(x, dtype=np.float32)
    W_kqv = np.asarray(W_kqv, dtype=np.float32)
    slopes = _alibi_slopes()

    nc = _get_nc()
    in_maps = []
    for c in range(N_CORES):
        b, hb = c // H_LOC, c % H_LOC
        # strided heads: local slot j -> global head hb + 4j. Slot j's slope
        # range is then uniform across cores, which makes the per-slot ALiBi
        # tile-skip thresholds in the (shared SPMD) graph valid everywhere.
        gh = [hb + H_LOC * j for j in range(H_LOC)]
        wk = np.concatenate([W_kqv[g * D:(g + 1) * D, :] for g in gh])
        wq = np.concatenate(
            [W_kqv[E + g * D:E + (g + 1) * D, :] for g in gh]) \
            * np.float32(SCALE)
        wv = np.concatenate(
            [W_kqv[2 * E + g * D:2 * E + (g + 1) * D, :] for g in gh])
        bk, bq = _bias_row_blocks(slopes[gh])
        import ml_dtypes
        in_maps.append({
            "xt": np.ascontiguousarray(x[b].T).astype(ml_dtypes.bfloat16),
            "wt_qk": np.ascontiguousarray(
                np.concatenate([wq, wk], axis=0).T).astype(ml_dtypes.bfloat16),
            "wt_v": np.ascontiguousarray(wv.T).astype(ml_dtypes.bfloat16),
            "slopes": np.tile(slopes[gh], (128, 1)),
            "brows_k": bk,
            "brows_q": bq,
        })

    if os.environ.get("BASS_NO_WARMUP") != "1":
        from concourse import bass2jax
        bass2jax.run_bass_via_pjrt(nc, in_maps, n_cores=N_CORES)

    res = run_bass_kernel_spmd(
        nc, in_maps, core_ids=list(range(N_CORES)),
        trace=os.environ.get("BASS_TRACE") == "1")

    outp = np.empty((B, S, E), dtype=np.float32)
    for c in range(N_CORES):
        b, hb = c // H_LOC, c % H_LOC
        co = res.results[c]["out"]
        for j in range(H_LOC):
            g = hb + H_LOC * j
            o = co[j * 65:j * 65 + 64, :]       # [d, q]
            den = co[j * 65 + 64:j * 65 + 65, :]  # [1, q]
            outp[b, :, g * D:(g + 1) * D] = (o / den).T
    if os.environ.get("BASS_TRACE") == "1":
        kernel.last_exec_time_ns = res.exec_time_ns
        kernel.last_results = res
    return outp



# revision 8
# speedup vs baseline: 1.1133x; 1.1133x over previous
"""ALiBi multi-head attention on 8 TRN2 NeuronCores.

Problem: x [2, 2048, 1024] fp32, W_kqv [3072, 1024] fp32 (row chunks k,q,v),
16 heads x 64 dim, causal + ALiBi, softmax scale = sqrt(1024) = 32.

Sharding (no collectives): core c takes batch c//4 and the STRIDED heads
{hb, hb+4, hb+8, hb+12} with hb = c%4. Striding makes each local head
slot's ALiBi slope range uniform across cores, so the shared SPMD graph
can skip score tiles whose ALiBi decay makes them negligible
(slope*distance >= 8, ~3e-4 of softmax mass).

Device design (per core):
- Host pre-transposes x and the per-head W column shards and pre-casts to
  bf16; the 1/32 score scale is folded into the Q weights.
- The full ALiBi bias slope*(j-i) for GLOBAL positions is folded into the
  score matmul as 8 exact bf16 rank-1 row pairs: j/i split hi/lo (8+3
  mantissa bits), slope split sH+sL; every product is exact in fp32 MACs,
  so exp needs no bias AP at all - one activation instr per tile.
- Scores are computed transposed, S^T[j, i], so softmax(j) runs along
  partitions: no max-subtraction needed and the 2048^2 probability matrix
  is never transposed.
- Diagonal tiles clip the causally-dead column prefix [0, 128d) out of
  both the score and PV matmuls; the 128-wide staircase is zero-filled by
  a GpSimd affine_select on the exp output.
- Projection runs e-outer (accumulate 8 chains across all PSUM banks in
  parallel) so the first matmul needs only e-tile 0 of x/W; input DMA
  issues are spread across 4 engine queues in arrival order.
- Attention runs as a rolling 4-stream scheduler with scores emitted one
  tile ahead of PVs (depth-1 software pipeline); remaining projection
  chains are interspersed as PE filler matched against the Scalar
  engine's exp load.
- The denominator comes from a ones column appended to V. Raw [65, 512]
  accumulators ship to DRAM as bf16; the host folds the normalize and the
  [d, q] -> [q, d] transpose into the gather.
"""

import math
import os
import sys

import numpy as np

for _p in ("/opt/trn_rl_repo",):
    if _p not in sys.path:
        sys.path.insert(0, _p)

B, S, E = 2, 2048, 1024
H, D = 16, 64
H_LOC = 4          # heads per core
COLS = H_LOC * D   # 256 output columns per core
SCALE = 1.0 / math.sqrt(E)
N_CORES = 8

NE = E // 128      # 8 e-tiles
NS = S // 512      # 4 s-chunks of 512
NST = S // 128     # 16 s-tiles of 128

# ALiBi tile-skip thresholds per local head slot (strided head assignment
# keeps slot slope ranges uniform across cores): drop (j,i) pairs with
# slope*(i-j) >= 8  ->  <~3.4e-4 of softmax mass.
DJ = [8 * (4 ** (j + 1)) for j in range(H_LOC)]

_NC_CACHE = [None]


def _kt_min(h, qc):
    for kt in range(16):
        if qc * 512 - kt * 128 - 127 < DJ[h]:
            return kt
    return 16


def _hi_clip(h, qc, kt):
    return min(512, kt * 128 + 127 + DJ[h] - qc * 512 + 1)


def _build():
    import concourse.bacc as bacc
    import concourse.mybir as mybir
    import concourse.tile as tile

    f32 = mybir.dt.float32
    bf16 = mybir.dt.bfloat16
    nc = bacc.Bacc("TRN2", target_bir_lowering=False, debug=False,
                   num_devices=N_CORES)

    xt = nc.dram_tensor("xt", [E, S], bf16, kind="ExternalInput")
    wt_qk = nc.dram_tensor("wt_qk", [E, 2 * COLS], bf16,
                           kind="ExternalInput")
    wt_v = nc.dram_tensor("wt_v", [E, COLS], bf16, kind="ExternalInput")
    brows_k = nc.dram_tensor("brows_k", [8 * H_LOC, S], bf16,
                             kind="ExternalInput")
    brows_q = nc.dram_tensor("brows_q", [8 * H_LOC, S], bf16,
                             kind="ExternalInput")
    out = nc.dram_tensor("out", [H_LOC * 65, S], bf16,
                         kind="ExternalOutput")

    with tile.TileContext(nc) as tc:
        with tc.tile_pool(name="const", bufs=1) as cpool, \
             tc.tile_pool(name="persist", bufs=1) as pp, \
             tc.tile_pool(name="inp", bufs=1) as ip, \
             tc.tile_pool(name="work", bufs=8) as wp, \
             tc.tile_pool(name="outb", bufs=4) as op:

            # ---- persistent activations ----
            # Per-head Q^T/K^T [128, S]: data in one 64-partition half
            # (h even: 0-63, h odd: 64-127), 8 exact ALiBi bias rows in
            # the padded half, rest zero (full K=128 contraction keeps
            # the PE activity monitor warm; zeros contribute nothing).
            qt = [pp.tile([128, S], bf16, tag=f"qt{h}", name=f"qt{h}")
                  for h in range(H_LOC)]
            kt_t = [pp.tile([128, S], bf16, tag=f"kt{h}", name=f"ktt{h}")
                    for h in range(H_LOC)]
            # V in [s, d] layout + ones column per head: one flat tile,
            # [128, 16 st x (4 heads x 65)].
            va = pp.tile([128, NST * H_LOC * 65], bf16, tag="va", name="va")

            dume = cpool.tile([1, 2], f32, tag="dume")

            xtr = [ip.tile([128, S], bf16, tag=f"xt{e}", name=f"xtr{e}")
                   for e in range(NE)]
            wqk = [ip.tile([128, 2 * COLS], bf16, tag=f"wqk{e}",
                           name=f"wqk{e}") for e in range(NE)]
            wv = [ip.tile([128, COLS], bf16, tag=f"wv{e}", name=f"wv{e}")
                  for e in range(NE)]

            # ---- preamble: exp-table preload + DMA issues + pad zeroing --
            # Dummy exp loads the Scalar engine's activation table during
            # the DMA window instead of gating the first real score tile.
            nc.gpsimd.memset(dume[:], 0.0)
            nc.scalar.activation(dume[:], dume[:],
                                 mybir.ActivationFunctionType.Exp)

            # Input DMA issues round-robin over the two HW-DGE engine
            # queues (Sync + Scalar) in arrival-need order.
            dmae = [nc.sync, nc.scalar]
            di = [0]

            def dma(dst, src):
                dmae[di[0] % 2].dma_start(dst, src)
                di[0] += 1

            for e in range(NE):
                dma(xtr[e][:], xt[e * 128:(e + 1) * 128, :])
                dma(wqk[e][:], wt_qk[e * 128:(e + 1) * 128, :])
            for e in range(NE):
                dma(wv[e][:], wt_v[e * 128:(e + 1) * 128, :])

            # Zero the non-data halves (bias rows land on top; Tile
            # orders the overlapping writes), then the ones column.
            for h in range(H_LOC):
                z0 = 64 if h % 2 == 0 else 0
                eng = nc.vector if h < 2 else nc.gpsimd
                eng.memset(qt[h][z0:z0 + 64, :], 0.0)
                eng.memset(kt_t[h][z0:z0 + 64, :], 0.0)
            vav = va[:].rearrange("p (t c) -> p t c", c=65)
            nc.vector.memset(vav[:, :, 64:65], 1.0)

            # Bias rows (issue after the memsets are emitted so Tile
            # orders memset -> rows; engines: sync/scalar only, whose
            # queues are otherwise free here).
            for h in range(H_LOC):
                br = 64 if h % 2 == 0 else 56
                nc.sync.dma_start(kt_t[h][br:br + 8, :],
                                  brows_k[8 * h:8 * h + 8, :])
                nc.scalar.dma_start(qt[h][br:br + 8, :],
                                    brows_q[8 * h:8 * h + 8, :])

            # ---- copy helpers (PSUM reads: Vector only - GpSimd has no
            # PSUM access, Scalar would thrash the activation table) ----
            def qk_copies(p, f):
                # psum half-row blocks go to the heads' padded tiles
                # (same partitions - engines cannot move data across
                # partitions).  f0 -> qt[0]/qt[1], f1 -> qt[2]/qt[3],
                # f2 -> kt[0]/kt[1], f3 -> kt[2]/kt[3].
                sc = f[1]
                sl = slice(sc * 512, (sc + 1) * 512)
                dst = qt if f[0] < 2 else kt_t
                fb = f[0] % 2
                nc.vector.tensor_copy(dst[2 * fb][0:64, sl], p[0:64, :])
                nc.vector.tensor_copy(dst[2 * fb + 1][64:128, sl],
                                      p[64:128, :])

            def v_copy(p, st):
                dst = va[:, st * 260:(st + 1) * 260].rearrange(
                    "p (h c) -> p h c", h=H_LOC)
                nc.vector.tensor_copy(
                    dst[:, :, 0:64],
                    p[:].rearrange("p (h c) -> p h c", h=H_LOC))

            # ---- phase 1: Q01^T/K01^T, e-outer over 8 PSUM banks ----
            with tc.tile_pool(name="ps8", bufs=8, space="PSUM") as ps8:
                fs = [(0, sc) for sc in range(NS)] + \
                     [(2, sc) for sc in range(NS)]
                pqk = {f: ps8.tile([128, 512], f32, tag="p", name="pqk")
                       for f in fs}
                for e in range(NE):
                    for f in fs:
                        nc.tensor.matmul(
                            pqk[f][:],
                            wqk[e][:, f[0] * 128:(f[0] + 1) * 128],
                            xtr[e][:, f[1] * 512:(f[1] + 1) * 512],
                            start=(e == 0), stop=(e == NE - 1))
                for f in fs:
                    qk_copies(pqk[f], f)

                # ---- phase 2: V tiles st 0-3 ----
                pv = {st: ps8.tile([128, COLS], f32, tag="p", name="pv",
                                   padded_shape=[128, 512])
                      for st in range(4)}
                for e in range(NE):
                    for st in range(4):
                        nc.tensor.matmul(
                            pv[st][:],
                            xtr[e][:, st * 128:(st + 1) * 128],
                            wv[e][:],
                            start=(e == 0), stop=(e == NE - 1))
                for st in range(4):
                    v_copy(pv[st], st)

            # ---- attention + interleaved projection filler ----
            with tc.tile_pool(name="ps_s", bufs=2, space="PSUM") as ps_s, \
                 tc.tile_pool(name="ps_o", bufs=4, space="PSUM") as ps_o, \
                 tc.tile_pool(name="ps_f", bufs=2, space="PSUM") as ps_f:

                def v_chain(st):
                    p = ps_f.tile([128, COLS], f32, tag="pf", name="pvf",
                                  padded_shape=[128, 512])
                    for e in range(NE):
                        nc.tensor.matmul(
                            p[:], xtr[e][:, st * 128:(st + 1) * 128],
                            wv[e][:], start=(e == 0), stop=(e == NE - 1))
                    v_copy(p, st)

                def f_chain(f):
                    p = ps_f.tile([128, 512], f32, tag="pf", name="pqf")
                    for e in range(NE):
                        nc.tensor.matmul(
                            p[:], wqk[e][:, f[0] * 128:(f[0] + 1) * 128],
                            xtr[e][:, f[1] * 512:(f[1] + 1) * 512],
                            start=(e == 0), stop=(e == NE - 1))
                    qk_copies(p, f)

                def emit_score(h, qc, kt):
                    lo = max(0, 128 * (kt - 4 * qc))
                    hic = _hi_clip(h, qc, kt)
                    ps = ps_s.tile([128, 512], f32, tag="s", name="ps")
                    nc.tensor.matmul(
                        ps[:, lo:hic],
                        kt_t[h][:, kt * 128:(kt + 1) * 128],
                        qt[h][:, qc * 512 + lo:qc * 512 + hic],
                        start=True, stop=True)
                    et = wp.tile([128, 512], bf16, tag="et", name="et")
                    nc.scalar.activation(
                        et[:, lo:hic], ps[:, lo:hic],
                        mybir.ActivationFunctionType.Exp)
                    if kt - 4 * qc >= 0:
                        # zero the causally-masked staircase (exp
                        # overflowed to +inf there; the fill never reads
                        # the inputs it masks)
                        nc.gpsimd.affine_select(
                            out=et[:, lo:lo + 128], in_=et[:, lo:lo + 128],
                            compare_op=mybir.AluOpType.is_ge,
                            fill=0.0, base=0, pattern=[[1, 128]],
                            channel_multiplier=-1)
                    return et, lo, hic

                def emit_pv(st_, kt, et, lo, hic):
                    h = st_["h"]
                    nc.tensor.matmul(
                        st_["po"][:, lo:hic],
                        va[:, kt * 260 + h * 65:kt * 260 + h * 65 + 65],
                        et[:, lo:hic],
                        start=(kt == st_["km"]),
                        stop=(kt == st_["ktmax"]))

                def epilogue(st_, last=False):
                    h, qc, po = st_["h"], st_["qc"], st_["po"]
                    if last:
                        # split the serial tail: copy+store per half, DMA
                        # halves on separate queues
                        for c in range(2):
                            osb = op.tile([65, 256], bf16, tag="osbh",
                                          name="osbh")
                            nc.vector.tensor_copy(
                                osb[:], po[:, c * 256:(c + 1) * 256])
                            eng = nc.sync if c == 0 else nc.scalar
                            eng.dma_start(
                                out[h * 65:(h + 1) * 65,
                                    qc * 512 + c * 256:
                                    qc * 512 + (c + 1) * 256],
                                osb[:])
                    else:
                        osb = op.tile([65, 512], bf16, tag="osb",
                                      name="osb")
                        nc.vector.tensor_copy(osb[:], po[:])
                        nc.sync.dma_start(
                            out[h * 65:(h + 1) * 65,
                                qc * 512:(qc + 1) * 512],
                            osb[:])

                def roll(init, queue, fillers):
                    """Rolling attention scheduler: up to 4 concurrent
                    (h, qc) streams (one PSUM accumulator each), scores
                    emitted one tile ahead of PVs, one filler chain per
                    round."""
                    queue = list(queue)
                    fillers = list(fillers)
                    live = []
                    # FIFO of deferred PV emissions: [delay_slots, fn].
                    # Normal tiles get 1 emission slot of slack, diagonal
                    # tiles 2 (their exp -> mask -> PV chain is longer;
                    # the score PSUM recycle only depends on exp, so the
                    # extra slot costs no PSUM).
                    pending = []

                    def age_and_flush():
                        for ent in pending:
                            ent[0] -= 1
                        while pending and pending[0][0] <= 0:
                            pending.pop(0)[1]()

                    def flush_all():
                        while pending:
                            pending.pop(0)[1]()

                    def activate(hqc):
                        h, qc = hqc
                        live.append({
                            "h": h, "qc": qc, "kt": _kt_min(h, qc),
                            "km": _kt_min(h, qc), "ktmax": 4 * qc + 3,
                            "po": ps_o.tile([65, 512], f32, tag="o",
                                            name=f"po{qc}{h}"),
                        })

                    for hqc in init:
                        activate(hqc)
                    while live:
                        emitted = False
                        for st_ in list(live):
                            kt = st_["kt"]
                            if kt > st_["ktmax"]:
                                continue
                            st_["kt"] = kt + 1
                            emitted = True
                            et, lo, hic = emit_score(st_["h"], st_["qc"],
                                                     kt)
                            age_and_flush()
                            done = kt == st_["ktmax"]

                            def mk(st_=st_, kt=kt, et=et, lo=lo, hic=hic,
                                   done=done):
                                emit_pv(st_, kt, et, lo, hic)
                                if done:
                                    live.remove(st_)
                                    is_last = (not live and not queue
                                               and not pending)
                                    epilogue(st_, last=is_last)
                                    if queue:
                                        activate(queue.pop(0))

                            diag = kt - 4 * st_["qc"] >= 0
                            pending.append([2 if diag else 1, mk])
                        if fillers and emitted:
                            fillers.pop(0)()
                        if not emitted or \
                                all(s["kt"] > s["ktmax"] for s in live):
                            flush_all()
                    flush_all()
                    for f in fillers:
                        f()

                # G1: head slots 0/1 x qc 0/1; filler: V st4-7 then the
                # slot-2/3 projection chains (f1 = Q23, f3 = K23).
                roll(
                    init=[(0, 0), (1, 0), (0, 1), (1, 1)],
                    queue=[],
                    fillers=[lambda st=st: v_chain(st)
                             for st in range(4, 8)] +
                            [lambda f=(fi, sc): f_chain(f)
                             for fi in (1, 3) for sc in range(NS)])

                # Main roll: everything else.  qc0/1 streams first (their
                # va/f deps are done), heavy qc2/3 streams next, light
                # ones last so the tail stays parallel; V st8-15 as
                # early-round filler.
                roll(
                    init=[(2, 1), (3, 1), (2, 0), (3, 0)],
                    queue=[(3, 3), (3, 2), (2, 2), (2, 3),
                           (0, 2), (1, 2), (0, 3), (1, 3)],
                    fillers=[lambda st=st: v_chain(st)
                             for st in range(8, 16)])

    nc.compile()
    return nc


def _get_nc():
    if _NC_CACHE[0] is None:
        _NC_CACHE[0] = _build()
    return _NC_CACHE[0]


def _alibi_slopes():
    x = (2 ** 8) ** (1.0 / H)
    return np.array([1.0 / x ** (i + 1) for i in range(H)], dtype=np.float32)


def _bias_row_blocks(slopes4: np.ndarray):
    """Exact bf16 bias rows: 8 per local head.

    bias[j, i] = slope*(j - i) encoded as rank-8 with hi/lo splits:
      k rows: [j_hi, j_lo, j_hi, j_lo, sH, sH, sL, sL]
      q rows: [sH, sH, sL, sL, -i_hi, -i_lo, -i_hi, -i_lo]
    j_hi = j & ~7 (8 mantissa bits), j_lo = j & 7, sH = bf16(slope),
    sL = bf16(slope - sH): every product is exact in fp32 MACs; the
    residual slope error is ~slope*2^-16 (bias error < 2e-4 at the skip
    threshold).
    """
    import ml_dtypes
    j = np.arange(S).astype(np.float32)
    j_hi = (np.arange(S) & ~7).astype(np.float32)
    j_lo = (np.arange(S) & 7).astype(np.float32)
    bk = np.zeros((8 * H_LOC, S), dtype=np.float32)
    bq = np.zeros((8 * H_LOC, S), dtype=np.float32)
    for h in range(H_LOC):
        sh = np.float32(ml_dtypes.bfloat16(slopes4[h]))
        sl = np.float32(ml_dtypes.bfloat16(np.float32(slopes4[h]) - sh))
        bk[8 * h + 0] = j_hi
        bk[8 * h + 1] = j_lo
        bk[8 * h + 2] = j_hi
        bk[8 * h + 3] = j_lo
        bk[8 * h + 4] = sh
        bk[8 * h + 5] = sh
        bk[8 * h + 6] = sl
        bk[8 * h + 7] = sl
        bq[8 * h + 0] = sh
        bq[8 * h + 1] = sh
        bq[8 * h + 2] = sl
        bq[8 * h + 3] = sl
        bq[8 * h + 4] = -j_hi
        bq[8 * h + 5] = -j_lo
        bq[8 * h + 6] = -j_hi
        bq[8 * h + 7] = -j_lo
    return (bk.astype(ml_dtypes.bfloat16), bq.astype(ml_dtypes.bfloat16))


def kernel(x: np.ndarray, W_kqv: np.ndarray) -> np.ndarray:
    from concourse.bass_utils import run_bass_kernel_spmd
    import ml_dtypes

    x = np.asarray(x, dtype=np.float32)
    W_kqv = np.asarray(W_kqv, dtype=np.float32)
    slopes = _alibi_slopes()

    nc = _get_nc()
    in_maps = []
    for c in range(N_CORES):
        b, hb = c // H_LOC, c % H_LOC
        # strided heads: local slot j -> global head hb + 4j, so slot
        # slope ranges (and the graph's per-slot ALiBi skip thresholds)
        # are uniform across cores.
        gh = [hb + H_LOC * j for j in range(H_LOC)]
        wk = np.concatenate([W_kqv[g * D:(g + 1) * D, :] for g in gh])
        wq = np.concatenate(
            [W_kqv[E + g * D:E + (g + 1) * D, :] for g in gh]) \
            * np.float32(SCALE)
        wv = np.concatenate(
            [W_kqv[2 * E + g * D:2 * E + (g + 1) * D, :] for g in gh])
        bk, bq = _bias_row_blocks(slopes[gh])
        in_maps.append({
            "xt": np.ascontiguousarray(x[b].T).astype(ml_dtypes.bfloat16),
            "wt_qk": np.ascontiguousarray(
                np.concatenate([wq, wk], axis=0).T).astype(ml_dtypes.bfloat16),
            "wt_v": np.ascontiguousarray(wv.T).astype(ml_dtypes.bfloat16),
            "brows_k": bk,
            "brows_q": bq,
        })

    if os.environ.get("BASS_NO_WARMUP") != "1":
        from concourse import bass2jax
        bass2jax.run_bass_via_pjrt(nc, in_maps, n_cores=N_CORES)

    res = run_bass_kernel_spmd(
        nc, in_maps, core_ids=list(range(N_CORES)),
        trace=os.environ.get("BASS_TRACE") == "1")

    outp = np.empty((B, S, E), dtype=np.float32)
    for c in range(N_CORES):
        b, hb = c // H_LOC, c % H_LOC
        co = np.asarray(res.results[c]["out"], dtype=np.float32)
        for j in range(H_LOC):
            g = hb + H_LOC * j
            o = co[j * 65:j * 65 + 64, :]         # [d, q]
            den = co[j * 65 + 64:j * 65 + 65, :]  # [1, q]
            outp[b, :, g * D:(g + 1) * D] = (o / den).T
    if os.environ.get("BASS_TRACE") == "1":
        kernel.last_exec_time_ns = res.exec_time_ns
        kernel.last_results = res
    return outp


# revision 17
# speedup vs baseline: 1.2124x; 1.0890x over previous
"""ALiBi multi-head attention on 8 TRN2 NeuronCores.

Problem: x [2, 2048, 1024] fp32, W_kqv [3072, 1024] fp32 (row chunks k,q,v),
16 heads x 64 dim, causal + ALiBi, softmax scale = sqrt(1024) = 32.

Sharding (no collectives): core c takes batch c//4 and the STRIDED heads
{hb, hb+4, hb+8, hb+12} with hb = c%4. Striding makes each local head
slot's ALiBi slope range uniform across cores, so the shared SPMD graph
can skip score tiles whose ALiBi decay makes them negligible
(slope*distance >= 8, ~3e-4 of softmax mass).

Device design (per core):
- Host pre-transposes x and the per-head W column shards and pre-casts to
  bf16; the 1/32 score scale is folded into the Q weights.
- The full ALiBi bias slope*(j-i) for GLOBAL positions is folded into the
  score matmul as 8 exact bf16 rank-1 row pairs: j/i split hi/lo (8+3
  mantissa bits), slope split sH+sL; every product is exact in fp32 MACs,
  so exp needs no bias AP at all - one activation instr per tile.
- Scores are computed transposed, S^T[j, i], so softmax(j) runs along
  partitions: no max-subtraction needed and the 2048^2 probability matrix
  is never transposed.
- Diagonal tiles clip the causally-dead column prefix [0, 128d) out of
  both the score and PV matmuls; the 128-wide staircase is zero-filled by
  a GpSimd affine_select on the exp output.
- Projection runs e-outer (accumulate 8 chains across all PSUM banks in
  parallel) so the first matmul needs only e-tile 0 of x/W; input DMA
  issues are spread across 4 engine queues in arrival order.
- Attention runs as a rolling 4-stream scheduler with scores emitted one
  tile ahead of PVs (depth-1 software pipeline); remaining projection
  chains are interspersed as PE filler matched against the Scalar
  engine's exp load.
- The denominator comes from a ones column appended to V. Raw [65, 512]
  accumulators ship to DRAM as bf16; the host folds the normalize and the
  [d, q] -> [q, d] transpose into the gather.
"""

import math
import os
import sys

import numpy as np

for _p in ("/opt/trn_rl_repo",):
    if _p not in sys.path:
        sys.path.insert(0, _p)

B, S, E = 2, 2048, 1024
H, D = 16, 64
H_LOC = 4          # heads per core
COLS = H_LOC * D   # 256 output columns per core
SCALE = 1.0 / math.sqrt(E)
N_CORES = 8

NE = E // 128      # 8 e-tiles
NS = S // 512      # 4 s-chunks of 512
NST = S // 128     # 16 s-tiles of 128

# ALiBi tile-skip thresholds per local head slot (strided head assignment
# keeps slot slope ranges uniform across cores): drop (j,i) pairs with
# slope*(i-j) >= 8  ->  <~3.4e-4 of softmax mass.
DJ = [8 * (4 ** (j + 1)) for j in range(H_LOC)]

_NC_CACHE = [None]


def _kt_min(h, qc):
    for kt in range(16):
        if qc * 512 - kt * 128 - 127 < DJ[h]:
            return kt
    return 16


def _hi_clip(h, qc, kt):
    return min(512, kt * 128 + 127 + DJ[h] - qc * 512 + 1)


def _build():
    import concourse.bacc as bacc
    import concourse.mybir as mybir
    import concourse.tile as tile

    f32 = mybir.dt.float32
    bf16 = mybir.dt.bfloat16
    nc = bacc.Bacc("TRN2", target_bir_lowering=False, debug=False,
                   num_devices=N_CORES)

    xt = nc.dram_tensor("xt", [E, S], bf16, kind="ExternalInput")
    wt_qk = nc.dram_tensor("wt_qk", [E, 2 * COLS], bf16,
                           kind="ExternalInput")
    wt_v = nc.dram_tensor("wt_v", [E, COLS], bf16, kind="ExternalInput")
    brows_k = nc.dram_tensor("brows_k", [8 * H_LOC, S], bf16,
                             kind="ExternalInput")
    brows_q = nc.dram_tensor("brows_q", [8 * H_LOC, S], bf16,
                             kind="ExternalInput")
    # one contiguous [65, 512] block per (h, qc) stream: cheap 1D-ish
    # epilogue DMAs
    out = nc.dram_tensor("out", [H_LOC * NS * 65, 512], bf16,
                         kind="ExternalOutput")

    with tile.TileContext(nc) as tc:
        with tc.tile_pool(name="const", bufs=1) as cpool, \
             tc.tile_pool(name="persist", bufs=1) as pp, \
             tc.tile_pool(name="inp", bufs=1) as ip, \
             tc.tile_pool(name="work", bufs=8) as wp, \
             tc.tile_pool(name="outb", bufs=4) as op:

            # ---- persistent activations ----
            # Per-head Q^T/K^T [128, S]: data in one 64-partition half
            # (h even: 0-63, h odd: 64-127), 8 exact ALiBi bias rows in
            # the padded half, rest zero (full K=128 contraction keeps
            # the PE activity monitor warm; zeros contribute nothing).
            qt = [pp.tile([128, S], bf16, tag=f"qt{h}", name=f"qt{h}")
                  for h in range(H_LOC)]
            kt_t = [pp.tile([128, S], bf16, tag=f"kt{h}", name=f"ktt{h}")
                    for h in range(H_LOC)]
            # V in [s, d] layout + ones column per head: one flat tile,
            # [128, 16 st x (4 heads x 65)].
            va = pp.tile([128, NST * H_LOC * 65], bf16, tag="va", name="va")

            dume = cpool.tile([1, 2], f32, tag="dume")

            xtr = [ip.tile([128, S], bf16, tag=f"xt{e}", name=f"xtr{e}")
                   for e in range(NE)]
            wqk = [ip.tile([128, 2 * COLS], bf16, tag=f"wqk{e}",
                           name=f"wqk{e}") for e in range(NE)]
            wv = [ip.tile([128, COLS], bf16, tag=f"wv{e}", name=f"wv{e}")
                  for e in range(NE)]

            # ---- preamble: exp-table preload + DMA issues + pad zeroing --
            # Dummy exp loads the Scalar engine's activation table during
            # the DMA window instead of gating the first real score tile.
            nc.gpsimd.memset(dume[:], 0.0)
            nc.scalar.activation(dume[:], dume[:],
                                 mybir.ActivationFunctionType.Exp)

            # Input DMA issues round-robin over the two HW-DGE engine
            # queues (Sync + Scalar) in arrival-need order: x column
            # halves h0 + W first (everything slots 0/1 x qc 0/1 needs),
            # then V weights, bias rows, and the x h1 halves, which
            # stream in while early attention computes.
            dmae = [nc.sync, nc.scalar]
            di = [0]

            def dma(dst, src):
                dmae[di[0] % 2].dma_start(dst, src)
                di[0] += 1

            for e in range(NE):
                dma(xtr[e][:, 0:1024], xt[e * 128:(e + 1) * 128, 0:1024])
                dma(wqk[e][:], wt_qk[e * 128:(e + 1) * 128, :])
            for e in range(NE):
                dma(wv[e][:], wt_v[e * 128:(e + 1) * 128, :])

            # Zero the non-data halves (bias rows land on top; Tile
            # orders the overlapping writes), then the ones column.
            for h in range(H_LOC):
                z0 = 64 if h % 2 == 0 else 0
                eng = nc.vector if h < 2 else nc.gpsimd
                eng.memset(qt[h][z0:z0 + 64, :], 0.0)
                eng.memset(kt_t[h][z0:z0 + 64, :], 0.0)
            vav = va[:].rearrange("p (t c) -> p t c", c=65)
            nc.vector.memset(vav[:, :, 64:65], 1.0)

            # Bias rows (issue after the memsets are emitted so Tile
            # orders memset -> rows).
            for h in range(H_LOC):
                br = 64 if h % 2 == 0 else 56
                dma(kt_t[h][br:br + 8, :], brows_k[8 * h:8 * h + 8, :])
                dma(qt[h][br:br + 8, :], brows_q[8 * h:8 * h + 8, :])
            for e in range(NE):
                dma(xtr[e][:, 1024:2048],
                    xt[e * 128:(e + 1) * 128, 1024:2048])

            # ---- copy helpers: PSUM reads go to Vector and Scalar
            # (GpSimd has no PSUM access; Scalar's Copy shares the
            # exp_and_others activation table with Exp, so no table
            # reload).
            def qk_copies(p, f):
                # psum half-row blocks go to the heads' padded tiles
                # (same partitions - engines cannot move data across
                # partitions).  f0 -> qt[0]/qt[1], f1 -> qt[2]/qt[3],
                # f2 -> kt[0]/kt[1], f3 -> kt[2]/kt[3].
                sc = f[1]
                sl = slice(sc * 512, (sc + 1) * 512)
                dst = qt if f[0] < 2 else kt_t
                fb = f[0] % 2
                nc.vector.tensor_copy(dst[2 * fb][0:64, sl], p[0:64, :])
                nc.scalar.copy(dst[2 * fb + 1][64:128, sl], p[64:128, :])

            vci = [0]

            def v_copy(p, st):
                dst = va[:, st * 260:(st + 1) * 260].rearrange(
                    "p (h c) -> p h c", h=H_LOC)
                src = p[:].rearrange("p (h c) -> p h c", h=H_LOC)
                if vci[0] % 2 == 0:
                    nc.vector.tensor_copy(dst[:, :, 0:64], src)
                else:
                    nc.scalar.copy(dst[:, :, 0:64], src)
                vci[0] += 1

            # ---- wave 1: Q01^T/K01^T for sc 0/1 (x half 0 only), then
            # V tiles st 0-7: exactly the dependency set of the first
            # attention group, so it starts while x half 1 still streams.
            with tc.tile_pool(name="ps8", bufs=8, space="PSUM") as ps8:
                fs = [(0, 0), (0, 1), (2, 0), (2, 1)]
                pqk = {f: ps8.tile([128, 512], f32, tag="p", name="pqk")
                       for f in fs}
                for e in range(NE):
                    for f in fs:
                        nc.tensor.matmul(
                            pqk[f][:],
                            wqk[e][:, f[0] * 128:(f[0] + 1) * 128],
                            xtr[e][:, f[1] * 512:(f[1] + 1) * 512],
                            start=(e == 0), stop=(e == NE - 1))
                for f in fs:
                    qk_copies(pqk[f], f)

                pv = {st: ps8.tile([128, COLS], f32, tag="p", name="pv",
                                   padded_shape=[128, 512])
                      for st in range(8)}
                for e in range(NE):
                    for st in range(8):
                        nc.tensor.matmul(
                            pv[st][:],
                            xtr[e][:, st * 128:(st + 1) * 128],
                            wv[e][:],
                            start=(e == 0), stop=(e == NE - 1))
                for st in range(8):
                    v_copy(pv[st], st)

            # ---- attention + interleaved projection filler ----
            with tc.tile_pool(name="ps_s", bufs=2, space="PSUM") as ps_s, \
                 tc.tile_pool(name="ps_o", bufs=4, space="PSUM") as ps_o, \
                 tc.tile_pool(name="ps_f", bufs=2, space="PSUM") as ps_f:

                def v_chain(st):
                    p = ps_f.tile([128, COLS], f32, tag="pf", name="pvf",
                                  padded_shape=[128, 512])
                    for e in range(NE):
                        nc.tensor.matmul(
                            p[:], xtr[e][:, st * 128:(st + 1) * 128],
                            wv[e][:], start=(e == 0), stop=(e == NE - 1))
                    v_copy(p, st)

                def f_chain(f):
                    p = ps_f.tile([128, 512], f32, tag="pf", name="pqf")
                    for e in range(NE):
                        nc.tensor.matmul(
                            p[:], wqk[e][:, f[0] * 128:(f[0] + 1) * 128],
                            xtr[e][:, f[1] * 512:(f[1] + 1) * 512],
                            start=(e == 0), stop=(e == NE - 1))
                    qk_copies(p, f)

                def emit_score(h, qc, kt):
                    lo = max(0, 128 * (kt - 4 * qc))
                    hic = _hi_clip(h, qc, kt)
                    ps = ps_s.tile([128, 512], f32, tag="s", name="ps")
                    nc.tensor.matmul(
                        ps[:, lo:hic],
                        kt_t[h][:, kt * 128:(kt + 1) * 128],
                        qt[h][:, qc * 512 + lo:qc * 512 + hic],
                        start=True, stop=True)
                    et = wp.tile([128, 512], bf16, tag="et", name="et")
                    nc.scalar.activation(
                        et[:, lo:hic], ps[:, lo:hic],
                        mybir.ActivationFunctionType.Exp)
                    if kt - 4 * qc >= 0:
                        # zero the causally-masked staircase (exp
                        # overflowed to +inf there; the fill never reads
                        # the inputs it masks)
                        nc.gpsimd.affine_select(
                            out=et[:, lo:lo + 128], in_=et[:, lo:lo + 128],
                            compare_op=mybir.AluOpType.is_ge,
                            fill=0.0, base=0, pattern=[[1, 128]],
                            channel_multiplier=-1)
                    return et, lo, hic

                def emit_pv(st_, kt, et, lo, hic):
                    h = st_["h"]
                    nc.tensor.matmul(
                        st_["po"][:, lo:hic],
                        va[:, kt * 260 + h * 65:kt * 260 + h * 65 + 65],
                        et[:, lo:hic],
                        start=(kt == st_["km"]),
                        stop=(kt == st_["ktmax"]))

                ei = [0]

                def epilogue(st_, last=False):
                    h, qc, po = st_["h"], st_["qc"], st_["po"]
                    s0 = (h * NS + qc) * 65
                    if last:
                        # split the serial tail: copy+store per
                        # partition-row half (contiguous dram chunks) on
                        # separate engines/queues
                        osb = op.tile([65, 512], bf16, tag="osb",
                                      name="osbh")
                        nc.vector.tensor_copy(osb[0:32, :], po[0:32, :])
                        nc.scalar.copy(osb[32:64, :], po[32:64, :])
                        nc.scalar.copy(osb[64:65, :], po[64:65, :])
                        nc.sync.dma_start(out[s0:s0 + 32, :], osb[0:32, :])
                        nc.gpsimd.dma_start(out[s0 + 32:s0 + 65, :],
                                            osb[32:65, :])
                    else:
                        osb = op.tile([65, 512], bf16, tag="osb",
                                      name="osb")
                        nc.vector.tensor_copy(osb[:], po[:])
                        deng = nc.sync if ei[0] % 2 == 0 else nc.gpsimd
                        ei[0] += 1
                        deng.dma_start(out[s0:s0 + 65, :], osb[:])

                def roll(init, queue, fillers):
                    """Rolling attention scheduler: up to 4 concurrent
                    (h, qc) streams (one PSUM accumulator each), scores
                    emitted one tile ahead of PVs, one filler chain per
                    round."""
                    queue = list(queue)
                    fillers = list(fillers)
                    live = []
                    # FIFO of deferred PV emissions: [delay_slots, fn].
                    # Normal tiles get 1 emission slot of slack, diagonal
                    # tiles 2 (their exp -> mask -> PV chain is longer;
                    # the score PSUM recycle only depends on exp, so the
                    # extra slot costs no PSUM).
                    pending = []

                    def age_and_flush():
                        for ent in pending:
                            ent[0] -= 1
                        while pending and pending[0][0] <= 0:
                            pending.pop(0)[1]()

                    def flush_all():
                        while pending:
                            pending.pop(0)[1]()

                    def activate(hqc):
                        h, qc = hqc
                        live.append({
                            "h": h, "qc": qc, "kt": _kt_min(h, qc),
                            "km": _kt_min(h, qc), "ktmax": 4 * qc + 3,
                            "po": ps_o.tile([65, 512], f32, tag="o",
                                            name=f"po{qc}{h}"),
                        })

                    for hqc in init:
                        activate(hqc)
                    while live:
                        emitted = False
                        for st_ in list(live):
                            kt = st_["kt"]
                            if kt > st_["ktmax"]:
                                continue
                            st_["kt"] = kt + 1
                            emitted = True
                            et, lo, hic = emit_score(st_["h"], st_["qc"],
                                                     kt)
                            age_and_flush()
                            done = kt == st_["ktmax"]

                            def mk(st_=st_, kt=kt, et=et, lo=lo, hic=hic,
                                   done=done):
                                emit_pv(st_, kt, et, lo, hic)
                                if done:
                                    live.remove(st_)
                                    is_last = (not live and not queue
                                               and not pending)
                                    epilogue(st_, last=is_last)
                                    if queue:
                                        activate(queue.pop(0))

                            diag = kt - 4 * st_["qc"] >= 0
                            pending.append([2 if diag else 1, mk])
                        if fillers and emitted:
                            fillers.pop(0)()
                        if not emitted or \
                                all(s["kt"] > s["ktmax"] for s in live):
                            flush_all()
                    flush_all()
                    for f in fillers:
                        f()

                # G1: head slots 0/1 x qc 0/1 (deps: wave-1 output only);
                # filler: the slot-2/3 projection chains for sc 0/1,
                # which the main roll's initial streams need.
                roll(
                    init=[(0, 0), (1, 0), (0, 1), (1, 1)],
                    queue=[],
                    fillers=[lambda f=(fi, sc): f_chain(f)
                             for fi in (1, 3) for sc in (0, 1)])

                # Main roll: everything else.  qc0/1 streams first (deps
                # ready), heavy qc2/3 streams next, light ones last so
                # the tail stays parallel.  Fillers ordered by first
                # use: Q23/K23 sc2/3 chains before the qc2/3 entrants,
                # V st8-15 before their kt rounds, slot-0/1 sc2/3
                # chains before the late light streams.
                roll(
                    init=[(2, 1), (3, 1), (2, 0), (3, 0)],
                    queue=[(3, 3), (3, 2), (2, 2), (2, 3),
                           (0, 2), (1, 2), (0, 3), (1, 3)],
                    fillers=[lambda f=f: f_chain(f)
                             for f in ((1, 3), (1, 2), (3, 2))] +
                            [lambda: v_chain(8)] +
                            [lambda f=(3, 3): f_chain(f)] +
                            [lambda st=st: v_chain(st)
                             for st in range(9, 16)] +
                            [lambda f=f: f_chain(f)
                             for f in ((0, 2), (2, 2), (0, 3), (2, 3))])

    nc.compile()
    return nc


def _get_nc():
    if _NC_CACHE[0] is None:
        _NC_CACHE[0] = _build()
    return _NC_CACHE[0]


def _alibi_slopes():
    x = (2 ** 8) ** (1.0 / H)
    return np.array([1.0 / x ** (i + 1) for i in range(H)], dtype=np.float32)


def _bias_row_blocks(slopes4: np.ndarray):
    """Exact bf16 bias rows: 8 per local head.

    bias[j, i] = slope*(j - i) encoded as rank-8 with hi/lo splits:
      k rows: [j_hi, j_lo, j_hi, j_lo, sH, sH, sL, sL]
      q rows: [sH, sH, sL, sL, -i_hi, -i_lo, -i_hi, -i_lo]
    j_hi = j & ~7 (8 mantissa bits), j_lo = j & 7, sH = bf16(slope),
    sL = bf16(slope - sH): every product is exact in fp32 MACs; the
    residual slope error is ~slope*2^-16 (bias error < 2e-4 at the skip
    threshold).
    """
    import ml_dtypes
    j = np.arange(S).astype(np.float32)
    j_hi = (np.arange(S) & ~7).astype(np.float32)
    j_lo = (np.arange(S) & 7).astype(np.float32)
    bk = np.zeros((8 * H_LOC, S), dtype=np.float32)
    bq = np.zeros((8 * H_LOC, S), dtype=np.float32)
    for h in range(H_LOC):
        sh = np.float32(ml_dtypes.bfloat16(slopes4[h]))
        sl = np.float32(ml_dtypes.bfloat16(np.float32(slopes4[h]) - sh))
        bk[8 * h + 0] = j_hi
        bk[8 * h + 1] = j_lo
        bk[8 * h + 2] = j_hi
        bk[8 * h + 3] = j_lo
        bk[8 * h + 4] = sh
        bk[8 * h + 5] = sh
        bk[8 * h + 6] = sl
        bk[8 * h + 7] = sl
        bq[8 * h + 0] = sh
        bq[8 * h + 1] = sh
        bq[8 * h + 2] = sl
        bq[8 * h + 3] = sl
        bq[8 * h + 4] = -j_hi
        bq[8 * h + 5] = -j_lo
        bq[8 * h + 6] = -j_hi
        bq[8 * h + 7] = -j_lo
    return (bk.astype(ml_dtypes.bfloat16), bq.astype(ml_dtypes.bfloat16))


def kernel(x: np.ndarray, W_kqv: np.ndarray) -> np.ndarray:
    from concourse.bass_utils import run_bass_kernel_spmd
    import ml_dtypes

    x = np.asarray(x, dtype=np.float32)
    W_kqv = np.asarray(W_kqv, dtype=np.float32)
    slopes = _alibi_slopes()

    nc = _get_nc()
    in_maps = []
    for c in range(N_CORES):
        b, hb = c // H_LOC, c % H_LOC
        # strided heads: local slot j -> global head hb + 4j, so slot
        # slope ranges (and the graph's per-slot ALiBi skip thresholds)
        # are uniform across cores.
        gh = [hb + H_LOC * j for j in range(H_LOC)]
        wk = np.concatenate([W_kqv[g * D:(g + 1) * D, :] for g in gh])
        wq = np.concatenate(
            [W_kqv[E + g * D:E + (g + 1) * D, :] for g in gh]) \
            * np.float32(SCALE)
        wv = np.concatenate(
            [W_kqv[2 * E + g * D:2 * E + (g + 1) * D, :] for g in gh])
        bk, bq = _bias_row_blocks(slopes[gh])
        in_maps.append({
            "xt": np.ascontiguousarray(x[b].T).astype(ml_dtypes.bfloat16),
            "wt_qk": np.ascontiguousarray(
                np.concatenate([wq, wk], axis=0).T).astype(ml_dtypes.bfloat16),
            "wt_v": np.ascontiguousarray(wv.T).astype(ml_dtypes.bfloat16),
            "brows_k": bk,
            "brows_q": bq,
        })

    if os.environ.get("BASS_NO_WARMUP") != "1":
        from concourse import bass2jax
        bass2jax.run_bass_via_pjrt(nc, in_maps, n_cores=N_CORES)

    res = run_bass_kernel_spmd(
        nc, in_maps, core_ids=list(range(N_CORES)),
        trace=os.environ.get("BASS_TRACE") == "1")

    outp = np.empty((B, S, E), dtype=np.float32)
    for c in range(N_CORES):
        b, hb = c // H_LOC, c % H_LOC
        co = np.asarray(res.results[c]["out"], dtype=np.float32)
        for j in range(H_LOC):
            g = hb + H_LOC * j
            for qc in range(NS):
                s0 = (j * NS + qc) * 65
                o = co[s0:s0 + 64, :]        # [d, 512]
                den = co[s0 + 64:s0 + 65, :]
                outp[b, qc * 512:(qc + 1) * 512,
                     g * D:(g + 1) * D] = (o / den).T
    if os.environ.get("BASS_TRACE") == "1":
        kernel.last_exec_time_ns = res.exec_time_ns
        kernel.last_results = res
    return outp


# revision 20
# speedup vs baseline: 1.2634x; 1.0421x over previous
"""ALiBi multi-head attention on 8 TRN2 NeuronCores.

Problem: x [2, 2048, 1024] fp32, W_kqv [3072, 1024] fp32 (row chunks k,q,v),
16 heads x 64 dim, causal + ALiBi, softmax scale = sqrt(1024) = 32.

Sharding (no collectives): core c takes batch c//4 and the STRIDED heads
{hb, hb+4, hb+8, hb+12} with hb = c%4. Striding makes each local head
slot's ALiBi slope range uniform across cores, so the shared SPMD graph
can skip score tiles whose ALiBi decay makes them negligible
(slope*distance >= 8, ~3e-4 of softmax mass).

Device design (per core):
- Host pre-transposes x and the per-head W column shards and pre-casts to
  bf16; the 1/32 score scale is folded into the Q weights.
- The full ALiBi bias slope*(j-i) for GLOBAL positions is folded into the
  score matmul as 8 exact bf16 rank-1 row pairs: j/i split hi/lo (8+3
  mantissa bits), slope split sH+sL; every product is exact in fp32 MACs,
  so exp needs no bias AP at all - one activation instr per tile.
- Scores are computed transposed, S^T[j, i], so softmax(j) runs along
  partitions: no max-subtraction needed and the 2048^2 probability matrix
  is never transposed.
- Diagonal tiles clip the causally-dead column prefix [0, 128d) out of
  both the score and PV matmuls; the 128-wide staircase is zero-filled by
  a GpSimd affine_select on the exp output.
- Projection runs e-outer (accumulate 8 chains across all PSUM banks in
  parallel) so the first matmul needs only e-tile 0 of x/W; input DMA
  issues are spread across 4 engine queues in arrival order.
- Attention runs as a rolling 4-stream scheduler with scores emitted one
  tile ahead of PVs (depth-1 software pipeline); remaining projection
  chains are interspersed as PE filler matched against the Scalar
  engine's exp load.
- The denominator comes from a ones column appended to V. Raw [65, 512]
  accumulators ship to DRAM as bf16; the host folds the normalize and the
  [d, q] -> [q, d] transpose into the gather.
"""

import math
import os
import sys

import numpy as np

for _p in ("/opt/trn_rl_repo",):
    if _p not in sys.path:
        sys.path.insert(0, _p)

B, S, E = 2, 2048, 1024
H, D = 16, 64
H_LOC = 4          # heads per core
COLS = H_LOC * D   # 256 output columns per core
SCALE = 1.0 / math.sqrt(E)
N_CORES = 8

NE = E // 128      # 8 e-tiles
NS = S // 512      # 4 s-chunks of 512
NST = S // 128     # 16 s-tiles of 128

# ALiBi tile-skip thresholds per local head slot (strided head assignment
# keeps slot slope ranges uniform across cores): drop (j,i) pairs with
# slope*(i-j) >= 8  ->  <~3.4e-4 of softmax mass.
DJ = [8 * (4 ** (j + 1)) for j in range(H_LOC)]

_NC_CACHE = [None]


def _kt_min(h, qc):
    for kt in range(16):
        if qc * 512 - kt * 128 - 127 < DJ[h]:
            return kt
    return 16


def _hi_clip(h, qc, kt):
    return min(512, kt * 128 + 127 + DJ[h] - qc * 512 + 1)


def _build():
    import concourse.bacc as bacc
    import concourse.mybir as mybir
    import concourse.tile as tile

    f32 = mybir.dt.float32
    bf16 = mybir.dt.bfloat16
    nc = bacc.Bacc("TRN2", target_bir_lowering=False, debug=False,
                   num_devices=N_CORES)

    xt = nc.dram_tensor("xt", [E, S], bf16, kind="ExternalInput")
    wt_qk = nc.dram_tensor("wt_qk", [E, 2 * COLS], bf16,
                           kind="ExternalInput")
    wt_v = nc.dram_tensor("wt_v", [E, COLS], bf16, kind="ExternalInput")
    brows_k = nc.dram_tensor("brows_k", [8 * H_LOC, S], bf16,
                             kind="ExternalInput")
    brows_q = nc.dram_tensor("brows_q", [8 * H_LOC, S], bf16,
                             kind="ExternalInput")
    # one contiguous [65, 512] block per (h, qc) stream: cheap 1D-ish
    # epilogue DMAs
    out = nc.dram_tensor("out", [H_LOC * NS * 65, 512], bf16,
                         kind="ExternalOutput")

    with tile.TileContext(nc) as tc:
        with tc.tile_pool(name="const", bufs=1) as cpool, \
             tc.tile_pool(name="persist", bufs=1) as pp, \
             tc.tile_pool(name="inp", bufs=1) as ip, \
             tc.tile_pool(name="work", bufs=8) as wp, \
             tc.tile_pool(name="outb", bufs=4) as op:

            # ---- persistent activations ----
            # Per-head Q^T/K^T [128, S]: data in one 64-partition half
            # (h even: 0-63, h odd: 64-127), 8 exact ALiBi bias rows in
            # the padded half, rest zero (full K=128 contraction keeps
            # the PE activity monitor warm; zeros contribute nothing).
            qt = [pp.tile([128, S], bf16, tag=f"qt{h}", name=f"qt{h}")
                  for h in range(H_LOC)]
            kt_t = [pp.tile([128, S], bf16, tag=f"kt{h}", name=f"ktt{h}")
                    for h in range(H_LOC)]
            # V in [s, d] layout + ones column per head: one flat tile,
            # [128, 16 st x (4 heads x 65)].
            va = pp.tile([128, NST * H_LOC * 65], bf16, tag="va", name="va")

            dume = cpool.tile([1, 2], f32, tag="dume")

            xtr = [ip.tile([128, S], bf16, tag=f"xt{e}", name=f"xtr{e}")
                   for e in range(NE)]
            wqk = [ip.tile([128, 2 * COLS], bf16, tag=f"wqk{e}",
                           name=f"wqk{e}") for e in range(NE)]
            wv = [ip.tile([128, COLS], bf16, tag=f"wv{e}", name=f"wv{e}")
                  for e in range(NE)]

            # ---- preamble: exp-table preload + DMA issues + pad zeroing --
            # Dummy exp loads the Scalar engine's activation table during
            # the DMA window instead of gating the first real score tile.
            nc.gpsimd.memset(dume[:], 0.0)
            nc.scalar.activation(dume[:], dume[:],
                                 mybir.ActivationFunctionType.Exp)

            # Input DMA issues round-robin over the two HW-DGE engine
            # queues (Sync + Scalar) in arrival-need order: x column
            # halves h0 + W first (everything slots 0/1 x qc 0/1 needs),
            # then V weights, bias rows, and the x h1 halves, which
            # stream in while early attention computes.
            dmae = [nc.sync, nc.scalar]
            di = [0]

            def dma(dst, src):
                dmae[di[0] % 2].dma_start(dst, src)
                di[0] += 1

            for e in range(NE):
                dma(xtr[e][:, 0:1024], xt[e * 128:(e + 1) * 128, 0:1024])
                dma(wqk[e][:], wt_qk[e * 128:(e + 1) * 128, :])
            for e in range(NE):
                dma(wv[e][:], wt_v[e * 128:(e + 1) * 128, :])

            # Zero the non-data halves (bias rows land on top; Tile
            # orders the overlapping writes), then the ones column.
            for h in range(H_LOC):
                z0 = 64 if h % 2 == 0 else 0
                eng = nc.vector if h < 2 else nc.gpsimd
                eng.memset(qt[h][z0:z0 + 64, :], 0.0)
                eng.memset(kt_t[h][z0:z0 + 64, :], 0.0)
            vav = va[:].rearrange("p (t c) -> p t c", c=65)
            nc.vector.memset(vav[:, :, 64:65], 1.0)

            # Bias rows and the x h1 halves all go to Sync (idle from
            # here on), freeing Scalar's queue for the wave-1 copies.
            for h in range(H_LOC):
                br = 64 if h % 2 == 0 else 56
                nc.sync.dma_start(kt_t[h][br:br + 8, :],
                                  brows_k[8 * h:8 * h + 8, :])
                nc.sync.dma_start(qt[h][br:br + 8, :],
                                  brows_q[8 * h:8 * h + 8, :])
            for e in range(NE):
                nc.sync.dma_start(xtr[e][:, 1024:2048],
                                  xt[e * 128:(e + 1) * 128, 1024:2048])

            # ---- copy helpers: PSUM reads go to Vector and Scalar
            # (GpSimd has no PSUM access; Scalar's Copy shares the
            # exp_and_others activation table with Exp, so no table
            # reload).
            def qk_copies(p, f):
                # psum half-row blocks go to the heads' padded tiles
                # (same partitions - engines cannot move data across
                # partitions).  f0 -> qt[0]/qt[1], f1 -> qt[2]/qt[3],
                # f2 -> kt[0]/kt[1], f3 -> kt[2]/kt[3].
                sc = f[1]
                sl = slice(sc * 512, (sc + 1) * 512)
                dst = qt if f[0] < 2 else kt_t
                fb = f[0] % 2
                nc.vector.tensor_copy(dst[2 * fb][0:64, sl], p[0:64, :])
                nc.scalar.copy(dst[2 * fb + 1][64:128, sl], p[64:128, :])

            vci = [0]

            def v_copy(p, st):
                dst = va[:, st * 260:(st + 1) * 260].rearrange(
                    "p (h c) -> p h c", h=H_LOC)
                src = p[:].rearrange("p (h c) -> p h c", h=H_LOC)
                if vci[0] % 2 == 0:
                    nc.vector.tensor_copy(dst[:, :, 0:64], src)
                else:
                    nc.scalar.copy(dst[:, :, 0:64], src)
                vci[0] += 1

            # ---- wave 1: Q01^T/K01^T for sc 0/1 (x half 0 only), then
            # V tiles st 0-7: exactly the dependency set of the first
            # attention group, so it starts while x half 1 still streams.
            with tc.tile_pool(name="ps8", bufs=8, space="PSUM") as ps8:
                fs = [(0, 0), (0, 1), (2, 0), (2, 1)]
                pqk = {f: ps8.tile([128, 512], f32, tag="p", name="pqk")
                       for f in fs}
                for e in range(NE):
                    for f in fs:
                        nc.tensor.matmul(
                            pqk[f][:],
                            wqk[e][:, f[0] * 128:(f[0] + 1) * 128],
                            xtr[e][:, f[1] * 512:(f[1] + 1) * 512],
                            start=(e == 0), stop=(e == NE - 1))
                for f in fs:
                    qk_copies(pqk[f], f)

                pv = {st: ps8.tile([128, COLS], f32, tag="p", name="pv",
                                   padded_shape=[128, 512])
                      for st in range(8)}
                for e in range(NE):
                    for st in range(8):
                        nc.tensor.matmul(
                            pv[st][:],
                            xtr[e][:, st * 128:(st + 1) * 128],
                            wv[e][:],
                            start=(e == 0), stop=(e == NE - 1))
                for st in range(8):
                    v_copy(pv[st], st)

            # ---- attention + interleaved projection filler ----
            with tc.tile_pool(name="ps_s", bufs=2, space="PSUM") as ps_s, \
                 tc.tile_pool(name="ps_o", bufs=4, space="PSUM") as ps_o, \
                 tc.tile_pool(name="ps_f", bufs=2, space="PSUM") as ps_f:

                def v_chain(st):
                    p = ps_f.tile([128, COLS], f32, tag="pf", name="pvf",
                                  padded_shape=[128, 512])
                    for e in range(NE):
                        nc.tensor.matmul(
                            p[:], xtr[e][:, st * 128:(st + 1) * 128],
                            wv[e][:], start=(e == 0), stop=(e == NE - 1))
                    v_copy(p, st)

                def f_chain(f):
                    p = ps_f.tile([128, 512], f32, tag="pf", name="pqf")
                    for e in range(NE):
                        nc.tensor.matmul(
                            p[:], wqk[e][:, f[0] * 128:(f[0] + 1) * 128],
                            xtr[e][:, f[1] * 512:(f[1] + 1) * 512],
                            start=(e == 0), stop=(e == NE - 1))
                    qk_copies(p, f)

                def emit_score(h, qc, kt):
                    lo = max(0, 128 * (kt - 4 * qc))
                    hic = _hi_clip(h, qc, kt)
                    ps = ps_s.tile([128, 512], f32, tag="s", name="ps")
                    nc.tensor.matmul(
                        ps[:, lo:hic],
                        kt_t[h][:, kt * 128:(kt + 1) * 128],
                        qt[h][:, qc * 512 + lo:qc * 512 + hic],
                        start=True, stop=True)
                    et = wp.tile([128, 512], bf16, tag="et", name="et")
                    nc.scalar.activation(
                        et[:, lo:hic], ps[:, lo:hic],
                        mybir.ActivationFunctionType.Exp)
                    if kt - 4 * qc >= 0:
                        # zero the causally-masked staircase (exp
                        # overflowed to +inf there; the fill never reads
                        # the inputs it masks)
                        nc.gpsimd.affine_select(
                            out=et[:, lo:lo + 128], in_=et[:, lo:lo + 128],
                            compare_op=mybir.AluOpType.is_ge,
                            fill=0.0, base=0, pattern=[[1, 128]],
                            channel_multiplier=-1)
                    return et, lo, hic

                def emit_pv(st_, kt, et, lo, hic):
                    h = st_["h"]
                    nc.tensor.matmul(
                        st_["po"][:, lo:hic],
                        va[:, kt * 260 + h * 65:kt * 260 + h * 65 + 65],
                        et[:, lo:hic],
                        start=(kt == st_["km"]),
                        stop=(kt == st_["ktmax"]))

                ei = [0]

                def epilogue(st_, last=False, tail=False):
                    h, qc, po = st_["h"], st_["qc"], st_["po"]
                    s0 = (h * NS + qc) * 65
                    if last:
                        # split the serial tail: parallel copies on
                        # vector+scalar, stores on both HW-DGE queues
                        osb = op.tile([65, 512], bf16, tag="osb",
                                      name="osbh")
                        nc.vector.tensor_copy(osb[0:64, :], po[0:64, :])
                        nc.scalar.copy(osb[64:65, :], po[64:65, :])
                        nc.sync.dma_start(out[s0:s0 + 64, :],
                                          osb[0:64, :])
                        nc.scalar.dma_start(out[s0 + 64:s0 + 65, :],
                                            osb[64:65, :])
                    else:
                        osb = op.tile([65, 512], bf16, tag="osb",
                                      name="osb")
                        nc.vector.tensor_copy(osb[:], po[:])
                        # while streams remain queued, Sync alone keeps
                        # up; in the end cluster alternate with Scalar
                        # (whose exp stream is draining by then)
                        deng = nc.sync
                        if tail:
                            deng = nc.sync if ei[0] % 2 == 0 else nc.scalar
                            ei[0] += 1
                        deng.dma_start(out[s0:s0 + 65, :], osb[:])

                def roll(init, queue, fillers):
                    """Rolling attention scheduler: up to 4 concurrent
                    (h, qc) streams (one PSUM accumulator each), scores
                    emitted one tile ahead of PVs, one filler chain per
                    round."""
                    queue = list(queue)
                    fillers = list(fillers)
                    live = []
                    # FIFO of deferred PV emissions: [delay_slots, fn].
                    # Normal tiles get 1 emission slot of slack, diagonal
                    # tiles 2 (their exp -> mask -> PV chain is longer;
                    # the score PSUM recycle only depends on exp, so the
                    # extra slot costs no PSUM).
                    pending = []

                    def age_and_flush():
                        for ent in pending:
                            ent[0] -= 1
                        while pending and pending[0][0] <= 0:
                            pending.pop(0)[1]()

                    def flush_all():
                        while pending:
                            pending.pop(0)[1]()

                    def activate(hqc):
                        h, qc = hqc
                        live.append({
                            "h": h, "qc": qc, "kt": _kt_min(h, qc),
                            "km": _kt_min(h, qc), "ktmax": 4 * qc + 3,
                            "po": ps_o.tile([65, 512], f32, tag="o",
                                            name=f"po{qc}{h}"),
                        })

                    for hqc in init:
                        activate(hqc)
                    while live:
                        emitted = False
                        for st_ in list(live):
                            kt = st_["kt"]
                            if kt > st_["ktmax"]:
                                continue
                            st_["kt"] = kt + 1
                            emitted = True
                            et, lo, hic = emit_score(st_["h"], st_["qc"],
                                                     kt)
                            age_and_flush()
                            done = kt == st_["ktmax"]

                            def mk(st_=st_, kt=kt, et=et, lo=lo, hic=hic,
                                   done=done):
                                emit_pv(st_, kt, et, lo, hic)
                                if done:
                                    live.remove(st_)
                                    is_last = (not live and not queue
                                               and not pending)
                                    epilogue(st_, last=is_last,
                                             tail=not queue)
                                    if queue:
                                        activate(queue.pop(0))

                            diag = kt - 4 * st_["qc"] >= 0
                            pending.append([2 if diag else 1, mk])
                        if fillers and emitted:
                            fillers.pop(0)()
                        if not emitted or \
                                all(s["kt"] > s["ktmax"] for s in live):
                            flush_all()
                    flush_all()
                    for f in fillers:
                        f()

                # G1: head slots 0/1 x qc 0/1 (deps: wave-1 output only);
                # filler: the slot-2/3 projection chains for sc 0/1,
                # which the main roll's initial streams need.
                roll(
                    init=[(0, 0), (1, 0), (0, 1), (1, 1)],
                    queue=[],
                    fillers=[lambda f=(fi, sc): f_chain(f)
                             for fi in (1, 3) for sc in (0, 1)])

                # Main roll: everything else.  qc0/1 streams first (deps
                # ready), heavy qc2/3 streams next, light ones last so
                # the tail stays parallel.  Fillers ordered by first
                # use: Q23/K23 sc2/3 chains before the qc2/3 entrants,
                # V st8-15 before their kt rounds, slot-0/1 sc2/3
                # chains before the late light streams.
                roll(
                    init=[(2, 1), (3, 1), (2, 0), (3, 0)],
                    queue=[(3, 3), (3, 2), (2, 2), (2, 3),
                           (0, 2), (1, 2), (0, 3), (1, 3)],
                    fillers=[lambda f=f: f_chain(f)
                             for f in ((1, 3), (1, 2), (3, 2))] +
                            [lambda: v_chain(8)] +
                            [lambda f=(3, 3): f_chain(f)] +
                            [lambda st=st: v_chain(st)
                             for st in range(9, 16)] +
                            [lambda f=f: f_chain(f)
                             for f in ((0, 2), (2, 2), (0, 3), (2, 3))])

    nc.compile()
    return nc


def _get_nc():
    if _NC_CACHE[0] is None:
        _NC_CACHE[0] = _build()
    return _NC_CACHE[0]


def _alibi_slopes():
    x = (2 ** 8) ** (1.0 / H)
    return np.array([1.0 / x ** (i + 1) for i in range(H)], dtype=np.float32)


def _bias_row_blocks(slopes4: np.ndarray):
    """Exact bf16 bias rows: 8 per local head.

    bias[j, i] = slope*(j - i) encoded as rank-8 with hi/lo splits:
      k rows: [j_hi, j_lo, j_hi, j_lo, sH, sH, sL, sL]
      q rows: [sH, sH, sL, sL, -i_hi, -i_lo, -i_hi, -i_lo]
    j_hi = j & ~7 (8 mantissa bits), j_lo = j & 7, sH = bf16(slope),
    sL = bf16(slope - sH): every product is exact in fp32 MACs; the
    residual slope error is ~slope*2^-16 (bias error < 2e-4 at the skip
    threshold).
    """
    import ml_dtypes
    j = np.arange(S).astype(np.float32)
    j_hi = (np.arange(S) & ~7).astype(np.float32)
    j_lo = (np.arange(S) & 7).astype(np.float32)
    bk = np.zeros((8 * H_LOC, S), dtype=np.float32)
    bq = np.zeros((8 * H_LOC, S), dtype=np.float32)
    for h in range(H_LOC):
        sh = np.float32(ml_dtypes.bfloat16(slopes4[h]))
        sl = np.float32(ml_dtypes.bfloat16(np.float32(slopes4[h]) - sh))
        bk[8 * h + 0] = j_hi
        bk[8 * h + 1] = j_lo
        bk[8 * h + 2] = j_hi
        bk[8 * h + 3] = j_lo
        bk[8 * h + 4] = sh
        bk[8 * h + 5] = sh
        bk[8 * h + 6] = sl
        bk[8 * h + 7] = sl
        bq[8 * h + 0] = sh
        bq[8 * h + 1] = sh
        bq[8 * h + 2] = sl
        bq[8 * h + 3] = sl
        bq[8 * h + 4] = -j_hi
        bq[8 * h + 5] = -j_lo
        bq[8 * h + 6] = -j_hi
        bq[8 * h + 7] = -j_lo
    return (bk.astype(ml_dtypes.bfloat16), bq.astype(ml_dtypes.bfloat16))


def kernel(x: np.ndarray, W_kqv: np.ndarray) -> np.ndarray:
    from concourse.bass_utils import run_bass_kernel_spmd
    import ml_dtypes

    x = np.asarray(x, dtype=np.float32)
    W_kqv = np.asarray(W_kqv, dtype=np.float32)
    slopes = _alibi_slopes()

    nc = _get_nc()
    in_maps = []
    for c in range(N_CORES):
        b, hb = c // H_LOC, c % H_LOC
        # strided heads: local slot j -> global head hb + 4j, so slot
        # slope ranges (and the graph's per-slot ALiBi skip thresholds)
        # are uniform across cores.
        gh = [hb + H_LOC * j for j in range(H_LOC)]
        wk = np.concatenate([W_kqv[g * D:(g + 1) * D, :] for g in gh])
        wq = np.concatenate(
            [W_kqv[E + g * D:E + (g + 1) * D, :] for g in gh]) \
            * np.float32(SCALE)
        wv = np.concatenate(
            [W_kqv[2 * E + g * D:2 * E + (g + 1) * D, :] for g in gh])
        bk, bq = _bias_row_blocks(slopes[gh])
        in_maps.append({
            "xt": np.ascontiguousarray(x[b].T).astype(ml_dtypes.bfloat16),
            "wt_qk": np.ascontiguousarray(
                np.concatenate([wq, wk], axis=0).T).astype(ml_dtypes.bfloat16),
            "wt_v": np.ascontiguousarray(wv.T).astype(ml_dtypes.bfloat16),
            "brows_k": bk,
            "brows_q": bq,
        })

    if os.environ.get("BASS_NO_WARMUP") != "1":
        from concourse import bass2jax
        bass2jax.run_bass_via_pjrt(nc, in_maps, n_cores=N_CORES)

    res = run_bass_kernel_spmd(
        nc, in_maps, core_ids=list(range(N_CORES)),
        trace=os.environ.get("BASS_TRACE") == "1")

    outp = np.empty((B, S, E), dtype=np.float32)
    for c in range(N_CORES):
        b, hb = c // H_LOC, c % H_LOC
        co = np.asarray(res.results[c]["out"], dtype=np.float32)
        for j in range(H_LOC):
            g = hb + H_LOC * j
            for qc in range(NS):
                s0 = (j * NS + qc) * 65
                o = co[s0:s0 + 64, :]        # [d, 512]
                den = co[s0 + 64:s0 + 65, :]
                outp[b, qc * 512:(qc + 1) * 512,
                     g * D:(g + 1) * D] = (o / den).T
    if os.environ.get("BASS_TRACE") == "1":
        kernel.last_exec_time_ns = res.exec_time_ns
        kernel.last_results = res
    return outp
